# revision 1
# baseline (speedup 1.0000x reference)
"""Distributed Trainium2 Bass kernel for causal multi-head attention with RoPE.

Reference computation (B=2, S=2048, E=1024, H=16, D=64, fp32):
    q = rope((x @ Wq.T).heads); k = rope((x @ Wk.T).heads); v = (x @ Wv.T).heads
    out = softmax(mask(q k^T / sqrt(E))) v  -> concat heads -> @ Wo.T

Sharding (8 NeuronCores): data parallel over B (2 groups of 4 cores),
tensor parallel over heads within each group (4 heads per core).
Each core computes QKV for its 4 heads, flash-style causal attention,
normalized attention output transposed (d x s). A chunked AllGather
(4-rank groups, one chunk per 512-column sq block) concatenates the
per-head attention outputs while later chunks still compute; every core
then computes a 256-column slice of the final Wo projection per chunk.

Host-side prep (per-core input shards):
  - x fed transposed (E,S) in bf16.
  - Wq/Wk rows permuted per head to de-interleave RoPE pairs (even dims
    first, odd dims second) so RoPE becomes the rotate-half form.
  - cos/sin tables and the 32-row swap matrix are precomputed constants.
"""

import os
import sys

sys.path.insert(0, "/opt/trn_rl_repo")

import numpy as np
import ml_dtypes

import concourse.bass as bass
import concourse.bacc as bacc
import concourse.mybir as mybir
import concourse.tile as tile
from concourse import bass_utils

B, S, E, H, D = 2, 2048, 1024, 16, 64
NCORES = 8
TP = 4                 # tensor-parallel group size
HPC = H // TP          # heads per core = 4
DQ = HPC * D           # per-core projection width = 256
ATTN_SCALE = 1.0 / float(np.sqrt(E))

FP32 = mybir.dt.float32
BF16 = mybir.dt.bfloat16

SQT = 512              # sq tile (free dim of S^T tiles)
SKB = 128              # sk block (partition dim of S^T tiles)
NSQT = S // SQT        # 4
NST16 = S // 128       # 16
NE = E // 128          # 8 contraction steps

REPLICA_GROUPS = [[0, 1, 2, 3], [4, 5, 6, 7]]

_CACHE = {}
LAST_RESULT = None


def build_nc():
    nc = bacc.Bacc(None, target_bir_lowering=False)

    xT = nc.declare_dram_parameter("xT", [E, S], BF16, isOutput=False)
    wqT = nc.declare_dram_parameter("wqT", [E, DQ], BF16, isOutput=False)
    wkT = nc.declare_dram_parameter("wkT", [E, DQ], BF16, isOutput=False)
    wvT = nc.declare_dram_parameter("wvT", [E, DQ], BF16, isOutput=False)
    woT = nc.declare_dram_parameter("woT", [E, DQ], BF16, isOutput=False)
    cosd = nc.declare_dram_parameter("cos", [128, S], FP32, isOutput=False)
    sind = nc.declare_dram_parameter("sin", [128, S], FP32, isOutput=False)
    swapd = nc.declare_dram_parameter("swapmat", [128, 128], BF16, isOutput=False)
    out_ext = nc.declare_dram_parameter("out", [S, DQ], FP32, isOutput=True)

    with tile.TileContext(nc) as tc:
        with (
            tc.tile_pool(name="dram", bufs=1, space="DRAM") as drampool,
            tc.tile_pool(name="const", bufs=1) as constpool,
        ):
            # ---- persistent SBUF tensors; DMA order gates pipeline start ----
            w_sb = {}
            for name in ("wq", "wk", "wv", "wo"):
                w_sb[name] = constpool.tile(
                    [128, NE * DQ], BF16, tag=f"w_{name}", name=f"w_{name}"
                )

            def load_w(name, dram):
                for j in range(NE):
                    nc.sync.dma_start(
                        out=w_sb[name][:, j * DQ:(j + 1) * DQ],
                        in_=dram[j * 128:(j + 1) * 128, :],
                    )

            cos_sb = constpool.tile([128, S], FP32, tag="cos")
            sin_sb = constpool.tile([128, S], FP32, tag="sin")
            swap_sb = constpool.tile([128, 128], BF16, tag="swap")

            qt_sb = [
                constpool.tile([128, S], BF16, tag=f"qt{g}", name=f"qt{g}")
                for g in range(2)
            ]
            kt_sb = [
                constpool.tile([128, S], BF16, tag=f"kt{g}", name=f"kt{g}")
                for g in range(2)
            ]
            vaug = [
                constpool.tile([128, HPC * 65], BF16, tag=f"vaug{i}", name=f"vaug{i}")
                for i in range(NST16)
            ]
            attnT = [
                constpool.tile([64, S], BF16, tag=f"attn{h}", name=f"attn{h}")
                for h in range(HPC)
            ]

            # ---------------- Phase 1: QKV projections + RoPE ----------------
            with (
                tc.tile_pool(name="xt", bufs=1) as xtpool,
                tc.tile_pool(name="ps1", bufs=2, space="PSUM") as ps1pool,
                tc.tile_pool(name="ps2", bufs=2, space="PSUM") as ps2pool,
                tc.tile_pool(name="psv", bufs=2, space="PSUM") as psvpool,
                tc.tile_pool(name="ropetmp", bufs=3) as rtpool,
            ):
                xt = [
                    xtpool.tile([128, S], BF16, tag=f"xT{j}", name=f"xT{j}")
                    for j in range(NE)
                ]
                # gate-critical loads first
                load_w("wq", wqT)
                for j in range(NE):
                    nc.sync.dma_start(out=xt[j][:], in_=xT[j * 128:(j + 1) * 128, :])
                load_w("wk", wkT)
                nc.sync.dma_start(out=swap_sb[:], in_=swapd[:])
                nc.sync.dma_start(out=cos_sb[:], in_=cosd[:])
                nc.sync.dma_start(out=sin_sb[:], in_=sind[:])
                load_w("wv", wvT)
                load_w("wo", woT)

                # V projection first: attention needs vaug tiles from kb=0,
                # so emitting V early keeps the attention pipeline unblocked.
                for i in range(NST16):
                    psv = psvpool.tile([128, DQ], FP32, tag="psv")
                    for j in range(NE):
                        nc.tensor.matmul(
                            psv[:],
                            lhsT=xt[j][:, i * 128:(i + 1) * 128],
                            rhs=w_sb["wv"][:, j * DQ:(j + 1) * DQ],
                            start=(j == 0),
                            stop=(j == NE - 1),
                        )
                    nc.gpsimd.memset(vaug[i][:], 1.0)
                    # one strided copy drops V into the 4 per-head 65-wide
                    # slots, leaving column 64 of each slot at 1.0
                    nc.scalar.copy(
                        vaug[i][:, 0:HPC * 65].rearrange(
                            "p (h w) -> p h w", h=HPC
                        )[:, :, 0:64],
                        psv[:].rearrange("p (h w) -> p h w", h=HPC),
                    )

                for g in range(2):
                    for st in range(NSQT):
                        sq = slice(st * SQT, (st + 1) * SQT)
                        for wname, dst in (("wq", qt_sb), ("wk", kt_sb)):
                            ps = ps1pool.tile([128, SQT], FP32, tag="ps")
                            for j in range(NE):
                                nc.tensor.matmul(
                                    ps[:],
                                    lhsT=w_sb[wname][
                                        :, j * DQ + g * 128: j * DQ + g * 128 + 128
                                    ],
                                    rhs=xt[j][:, sq],
                                    start=(j == 0),
                                    stop=(j == NE - 1),
                                )
                            raw = rtpool.tile([128, SQT], BF16, tag="raw")
                            nc.scalar.copy(raw[:], ps[:])
                            ps_sw = ps2pool.tile([128, SQT], FP32, tag="ps_sw")
                            nc.tensor.matmul(
                                ps_sw[:], lhsT=swap_sb[:], rhs=raw[:],
                                start=True, stop=True,
                            )
                            t1 = rtpool.tile([128, SQT], FP32, tag="t1")
                            nc.vector.tensor_mul(t1[:], ps_sw[:], sin_sb[:, sq])
                            t2 = rtpool.tile([128, SQT], FP32, tag="t2")
                            nc.vector.tensor_mul(t2[:], raw[:], cos_sb[:, sq])
                            nc.vector.tensor_add(dst[g][:, sq], t1[:], t2[:])

            # ------- Phase 2+3: causal attention, chunked AG, Wo -------
            with (
                tc.tile_pool(name="pss", bufs=2, space="PSUM") as psspool,
                tc.tile_pool(name="pso", bufs=1, space="PSUM") as psopool,
                tc.tile_pool(name="psw", bufs=2, space="PSUM") as pswpool,
                tc.tile_pool(name="pt", bufs=3) as ptpool,
                tc.tile_pool(name="fin", bufs=2) as finpool,
                tc.tile_pool(name="gt", bufs=2) as gtpool,
                tc.tile_pool(name="osb", bufs=3) as osbpool,
            ):
                # sq chunks; the last 512 block is split so the tail-exposed
                # final AllGather is half size. Parity psum slots always sit
                # at 0/512-f32 offsets (separate PSUM banks) — the two
                # concurrently-issued parity matmuls must never share a bank
                # (fatal PSUM collision).
                CHUNKS = [(0, 512), (512, 512), (1024, 512), (1536, 512)]

                def wo_block(sq0, cw, gt):
                    for i4 in range(cw // 128):
                        r0 = sq0 + i4 * 128
                        psw = pswpool.tile(
                            [128, DQ], FP32, tag="psw", name=f"psw{r0}"
                        )
                        for j in range(NE):
                            nc.tensor.matmul(
                                psw[:],
                                lhsT=gt[j][:, i4 * 128:(i4 + 1) * 128],
                                rhs=w_sb["wo"][:, j * DQ:(j + 1) * DQ],
                                start=(j == 0),
                                stop=(j == NE - 1),
                            )
                        osb = osbpool.tile(
                            [128, DQ], FP32, tag="osb", name=f"osb{r0}"
                        )
                        nc.vector.tensor_copy(osb[:], psw[:])
                        nc.sync.dma_start(
                            out=out_ext[r0:r0 + 128, :], in_=osb[:]
                        )

                wo_queue = []    # (sq0, cw, gt tiles) ready for Wo
                ag_pending = []  # (sq0, cw, agout) awaiting gt load
                for ci, (sq0, cw) in enumerate(CHUNKS):
                    sq = slice(sq0, sq0 + cw)
                    nblk = (sq0 + cw) // SKB
                    for g in range(2):
                        pso = [
                            psopool.tile([65, cw], FP32, tag=f"pso{p}",
                                         name=f"pso{p}_{g}_{ci}")
                            for p in range(2)
                        ]
                        for kb in range(nblk):
                            pss = psspool.tile([SKB, 2 * SQT], FP32, tag="pss",
                                               name=f"pss_{g}_{ci}_{kb}")
                            for p in range(2):
                                nc.tensor.matmul(
                                    pss[:, p * SQT: p * SQT + cw],
                                    lhsT=kt_sb[g][
                                        p * 64:(p + 1) * 64, kb * SKB:(kb + 1) * SKB
                                    ],
                                    rhs=qt_sb[g][p * 64:(p + 1) * 64, sq],
                                    start=True,
                                    stop=True,
                                )
                            pt = ptpool.tile([SKB, 2 * SQT], BF16, tag="pt",
                                             name=f"pt_{g}_{ci}_{kb}")
                            diag = (kb + 1) * SKB > sq0
                            if cw == SQT:
                                nc.scalar.activation(
                                    pt[:], pss[:],
                                    mybir.ActivationFunctionType.Exp,
                                    scale=ATTN_SCALE,
                                )
                                if diag:
                                    nc.gpsimd.affine_select(
                                        out=pt[:],
                                        in_=pt[:],
                                        compare_op=mybir.AluOpType.is_ge,
                                        fill=0.0,
                                        base=sq0 - kb * SKB,
                                        channel_multiplier=-1,
                                        pattern=[[0, 2], [1, cw]],
                                    )
                            else:
                                for p in range(2):
                                    sl = slice(p * SQT, p * SQT + cw)
                                    nc.scalar.activation(
                                        pt[:, sl], pss[:, sl],
                                        mybir.ActivationFunctionType.Exp,
                                        scale=ATTN_SCALE,
                                    )
                                    if diag:
                                        nc.gpsimd.affine_select(
                                            out=pt[:, sl],
                                            in_=pt[:, sl],
                                            compare_op=mybir.AluOpType.is_ge,
                                            fill=0.0,
                                            base=sq0 - kb * SKB,
                                            channel_multiplier=-1,
                                            pattern=[[1, cw]],
                                        )
                            for p in range(2):
                                h = 2 * g + p
                                nc.tensor.matmul(
                                    pso[p][:],
                                    lhsT=vaug[kb][:, h * 65:(h + 1) * 65],
                                    rhs=pt[:, p * SQT: p * SQT + cw],
                                    start=(kb == 0),
                                    stop=(kb == nblk - 1),
                                )
                        # evacuate both pso tiles FIRST so their PSUM slots
                        # free ~0.5us after the last PV — the slow reciprocal
                        # chain then runs off the PE-critical path.
                        un = []
                        lrow = []
                        for p in range(2):
                            u = finpool.tile([64, cw], FP32, tag=f"un{p}",
                                             name=f"un{p}_{g}_{ci}")
                            nc.vector.tensor_copy(u[:], pso[p][0:64, :])
                            lr = finpool.tile([1, cw], FP32, tag=f"lrow{p}",
                                              name=f"lrow{p}_{g}_{ci}")
                            nc.vector.tensor_copy(lr[:], pso[p][64:65, :])
                            un.append(u)
                            lrow.append(lr)
                        for p in range(2):
                            h = 2 * g + p
                            linv = finpool.tile([1, cw], FP32, tag=f"linv{p}")
                            nc.vector.reciprocal(linv[:], lrow[p][:])
                            lbc = finpool.tile([64, cw], FP32, tag=f"lbc{p}")
                            nc.gpsimd.partition_broadcast(lbc[:], linv[:])
                            nc.vector.tensor_mul(
                                attnT[h][:, sq], un[p][:], lbc[:]
                            )

                    # ---- AllGather this sq chunk ----
                    agin = drampool.tile(
                        [DQ, cw], BF16, tag=f"agin{ci}", name=f"agin{ci}"
                    )
                    agout = drampool.tile(
                        [E, cw], BF16, tag=f"agout{ci}", name=f"agout{ci}"
                    )
                    for h in range(HPC):
                        nc.sync.dma_start(
                            out=agin[h * 64:(h + 1) * 64, :], in_=attnT[h][:, sq]
                        )
                    nc.gpsimd.collective_compute(
                        "AllGather",
                        mybir.AluOpType.bypass,
                        ins=[agin.opt()],
                        outs=[agout.opt()],
                        replica_groups=REPLICA_GROUPS,
                    )
                    # Wo runs with a two-chunk lag and its gathered tiles are
                    # DMA'd one chunk after their AllGather was issued: the gt
                    # trigger's CC wait is then already satisfied, so it never
                    # clogs the in-order Sync queue (which would delay the
                    # next chunk's agin DMA and cascade AG delays).
                    if wo_queue:
                        wo_block(*wo_queue.pop(0))
                    if ag_pending:
                        psq0, pcw, pagout = ag_pending.pop(0)
                        gt = []
                        for j in range(NE):
                            t = gtpool.tile(
                                [128, pcw], BF16, tag=f"gt{j}",
                                name=f"gt{j}_{psq0}"
                            )
                            nc.sync.dma_start(
                                out=t[:], in_=pagout[j * 128:(j + 1) * 128, :]
                            )
                            gt.append(t)
                        wo_queue.append((psq0, pcw, gt))
                    ag_pending.append((sq0, cw, agout))
                while ag_pending:
                    psq0, pcw, pagout = ag_pending.pop(0)
                    gt = []
                    for j in range(NE):
                        t = gtpool.tile(
                            [128, pcw], BF16, tag=f"gt{j}", name=f"gt{j}_{psq0}"
                        )
                        nc.sync.dma_start(
                            out=t[:], in_=pagout[j * 128:(j + 1) * 128, :]
                        )
                        gt.append(t)
                    wo_queue.append((psq0, pcw, gt))
                while wo_queue:
                    wo_block(*wo_queue.pop(0))

    nc.finalize()
    return nc


def _host_tables():
    inv = 1.0 / (10000.0 ** (np.arange(0, D, 2, dtype=np.float64) / D))  # (32,)
    ang = np.arange(S, dtype=np.float64)[None, :] * inv[:, None]          # (32,S)
    cos32 = np.cos(ang)
    sin32 = np.sin(ang)
    cos = np.tile(cos32, (4, 1)).astype(np.float32)                       # (128,S)
    sin = np.concatenate([-sin32, sin32, -sin32, sin32], axis=0).astype(np.float32)
    swap = np.zeros((128, 128), np.float32)
    for k in range(128):
        blk = (k // 64) * 64
        swap[k, blk + ((k - blk) + 32) % 64] = 1.0
    return cos, sin, swap


def kernel(x, W_q, W_k, W_v, W_o):
    global LAST_RESULT
    if "nc" not in _CACHE:
        _CACHE["nc"] = build_nc()
    nc = _CACHE["nc"]

    bf = ml_dtypes.bfloat16
    perm = np.concatenate([np.arange(0, D, 2), np.arange(1, D, 2)])
    rowperm = (np.arange(H)[:, None] * D + perm[None, :]).reshape(-1)
    Wq_p = W_q[rowperm]
    Wk_p = W_k[rowperm]
    cos, sin, swap = _host_tables()
    swap_bf = swap.astype(bf)

    in_maps = []
    for c in range(NCORES):
        b, tp = c // TP, c % TP
        sl = slice(tp * DQ, (tp + 1) * DQ)
        in_maps.append({
            "xT": np.ascontiguousarray(x[b].T).astype(bf),
            "wqT": np.ascontiguousarray(Wq_p[sl].T).astype(bf),
            "wkT": np.ascontiguousarray(Wk_p[sl].T).astype(bf),
            "wvT": np.ascontiguousarray(W_v[sl].T).astype(bf),
            "woT": np.ascontiguousarray(W_o[sl].T).astype(bf),
            "cos": cos,
            "sin": sin,
            "swapmat": swap_bf,
        })

    res = bass_utils.run_bass_kernel_spmd(
        nc, in_maps, core_ids=list(range(NCORES)),
        tmpdir=os.environ.get("BASS_TMPDIR") or None,
    )
    LAST_RESULT = res
    out = np.empty((B, S, E), np.float32)
    for c in range(NCORES):
        b, tp = c // TP, c % TP
        out[b][:, tp * DQ:(tp + 1) * DQ] = np.asarray(
            res.results[c]["out"], dtype=np.float32
        )
    return out



# revision 6
# speedup vs baseline: 1.2877x; 1.2877x over previous
"""Distributed Trainium2 Bass kernel for causal multi-head attention with RoPE.

Reference computation (B=2, S=2048, E=1024, H=16, D=64, fp32):
    q = rope((x @ Wq.T).heads); k = rope((x @ Wk.T).heads); v = (x @ Wv.T).heads
    out = softmax(mask(q k^T / sqrt(E))) v  -> concat heads -> @ Wo.T

Sharding (8 NeuronCores): data parallel over B (2 groups of 4 cores),
tensor parallel over heads within each group (4 heads per core).
Each core computes QKV for its 4 heads, flash-style causal attention,
normalized attention output transposed (d x s), AllGathers per-512-col
chunks across its 4-rank group, then computes a 256-col slice of Wo.

Schedule notes (all queues are per-engine in-order):
  - QK projection for seq-block st is issued, then attention chunk st:
    exp (ACT) for chunk st overlaps the PE matmuls of block st+1.
  - The kb loop is software-pipelined: scores for kb+2 are issued before
    PV of kb, so the PE computes scores while ACT runs exp.
  - Diagonal score blocks are narrowed to the causal region; the
    remaining partial 128-col block is masked with one triangular
    constant via a DVE multiply.
  - All Wo blocks are issued after all attention+AG so the PE never
    stalls mid-stream on a collective; a tiny warm-up AllGather during
    phase 1 absorbs the collective-stream entry barrier.

Host-side prep (per-core input shards):
  - x fed transposed (E,S) in bf16.
  - Wq/Wk rows permuted per head to de-interleave RoPE pairs (even dims
    first, odd dims second) so RoPE becomes the rotate-half form.
  - cos/sin tables (bf16), the 32-row swap matrix and the [128,128]
    causal triangle mask are precomputed constants.
"""

import os
import sys

sys.path.insert(0, "/opt/trn_rl_repo")

import numpy as np
import ml_dtypes

import concourse.bass as bass
import concourse.bacc as bacc
import concourse.mybir as mybir
import concourse.tile as tile
from concourse import bass_utils

B, S, E, H, D = 2, 2048, 1024, 16, 64
NCORES = 8
TP = 4                 # tensor-parallel group size
HPC = H // TP          # heads per core = 4
DQ = HPC * D           # per-core projection width = 256
ATTN_SCALE = 1.0 / float(np.sqrt(E))

FP32 = mybir.dt.float32
BF16 = mybir.dt.bfloat16

SQT = 512              # sq chunk width
SKB = 128              # sk block (partition dim of score tiles)
NSQT = S // SQT        # 4
NST16 = S // 128       # 16
NE = E // 128          # 8 contraction steps

REPLICA_GROUPS = [[0, 1, 2, 3], [4, 5, 6, 7]]

_CACHE = {}
LAST_RESULT = None


def build_nc():
    nc = bacc.Bacc(None, target_bir_lowering=False)

    xT = nc.declare_dram_parameter("xT", [E, S], BF16, isOutput=False)
    wqT = nc.declare_dram_parameter("wqT", [E, DQ], BF16, isOutput=False)
    wkT = nc.declare_dram_parameter("wkT", [E, DQ], BF16, isOutput=False)
    wvT = nc.declare_dram_parameter("wvT", [E, DQ], BF16, isOutput=False)
    woT = nc.declare_dram_parameter("woT", [E, DQ], BF16, isOutput=False)
    cosd = nc.declare_dram_parameter("cos", [128, S], BF16, isOutput=False)
    sind = nc.declare_dram_parameter("sin", [128, S], BF16, isOutput=False)
    swapd = nc.declare_dram_parameter("swapmat", [128, 128], BF16, isOutput=False)
    maskd = nc.declare_dram_parameter("trimask", [128, 128], BF16, isOutput=False)
    out_ext = nc.declare_dram_parameter("out", [S, DQ], FP32, isOutput=True)

    with tile.TileContext(nc) as tc:
        with (
            tc.tile_pool(name="dram", bufs=1, space="DRAM") as drampool,
            tc.tile_pool(name="const", bufs=1) as constpool,
            tc.tile_pool(name="psA", bufs=2, space="PSUM") as psApool,
            tc.tile_pool(name="psO", bufs=1, space="PSUM") as psOpool,
            tc.tile_pool(name="psW", bufs=2, space="PSUM") as psWpool,
            tc.tile_pool(name="ptp", bufs=3) as ptpool,
            tc.tile_pool(name="rope", bufs=3) as rtpool,
            tc.tile_pool(name="fin", bufs=2) as finpool,
            tc.tile_pool(name="gtp", bufs=2) as gtpool,
            tc.tile_pool(name="osb", bufs=2) as osbpool,
        ):
            # ---- persistent SBUF tensors ----
            w_sb = {}
            for name in ("wq", "wk", "wv", "wo"):
                w_sb[name] = constpool.tile(
                    [128, NE * DQ], BF16, tag=f"w_{name}", name=f"w_{name}"
                )
            cos_sb = constpool.tile([128, S], BF16, tag="cos")
            sin_sb = constpool.tile([128, S], BF16, tag="sin")
            swap_sb = constpool.tile([128, 128], BF16, tag="swap")
            mask_sb = constpool.tile([128, 128], BF16, tag="mask")
            xt = [
                constpool.tile([128, S], BF16, tag=f"xT{j}", name=f"xT{j}")
                for j in range(NE)
            ]
            qt_sb = [
                constpool.tile([128, S], BF16, tag=f"qt{g}", name=f"qt{g}")
                for g in range(2)
            ]
            kt_sb = [
                constpool.tile([128, S], BF16, tag=f"kt{g}", name=f"kt{g}")
                for g in range(2)
            ]
            vaug = [
                constpool.tile([128, HPC * 65], BF16, tag=f"vaug{i}", name=f"vaug{i}")
                for i in range(NST16)
            ]
            # all 4 heads' attn^T concatenated along free dim: [:, h*S + s]
            attnT = constpool.tile([64, HPC * S], BF16, tag="attnT", name="attnT")

            def load_w(name, dram):
                # one DMA: w_sb[p, j*DQ + c] = dram[j*128 + p, c]
                nc.sync.dma_start(
                    out=w_sb[name][:].rearrange("p (j c) -> p j c", j=NE),
                    in_=dram.rearrange("(j p) c -> p j c", j=NE),
                )

            # ---------------- input loads (SP queue) ----------------
            load_w("wv", wvT)
            for j in range(NE):
                nc.sync.dma_start(out=xt[j][:], in_=xT[j * 128:(j + 1) * 128, :])
            load_w("wq", wqT)
            load_w("wk", wkT)
            nc.sync.dma_start(out=swap_sb[:], in_=swapd[:])
            nc.sync.dma_start(out=cos_sb[:], in_=cosd[:])
            nc.sync.dma_start(out=sin_sb[:], in_=sind[:])
            nc.sync.dma_start(out=mask_sb[:], in_=maskd[:])
            load_w("wo", woT)

            # ------------- gpsimd: vaug memsets, then warmup AG -------------
            for i in range(NST16):
                nc.gpsimd.memset(vaug[i][:], 1.0)
            warm_in = drampool.tile([64, 8], BF16, tag="warm_in", name="warm_in")
            warm_out = drampool.tile(
                [256, 8], BF16, tag="warm_out", name="warm_out",
            )
            nc.gpsimd.collective_compute(
                "AllGather",
                mybir.AluOpType.bypass,
                ins=[warm_in.opt()],
                outs=[warm_out.opt()],
                replica_groups=REPLICA_GROUPS,
            )

            # ---------------- V projection ----------------
            for i in range(NST16):
                psv = psWpool.tile([128, DQ], FP32, tag="psw", name=f"psv{i}")
                for j in range(NE):
                    nc.tensor.matmul(
                        psv[:],
                        lhsT=xt[j][:, i * 128:(i + 1) * 128],
                        rhs=w_sb["wv"][:, j * DQ:(j + 1) * DQ],
                        start=(j == 0),
                        stop=(j == NE - 1),
                    )
                # one strided copy drops V into the 4 per-head 65-wide
                # slots, leaving column 64 of each slot at 1.0
                nc.scalar.copy(
                    vaug[i][:, 0:HPC * 65].rearrange(
                        "p (h w) -> p h w", h=HPC
                    )[:, :, 0:64],
                    psv[:].rearrange("p (h w) -> p h w", h=HPC),
                )

            # ---------- interleaved QK projection+RoPE / attention ----------
            ag_bufs = []  # (agout, cw, sq0) in chunk order

            def qk_block(g, st):
                sq = slice(st * SQT, (st + 1) * SQT)
                for wname, dst in (("wq", qt_sb), ("wk", kt_sb)):
                    ps = psApool.tile(
                        [128, 2 * SQT], FP32, tag="psA", name=f"ps_{wname}{g}_{st}"
                    )
                    for j in range(NE):
                        nc.tensor.matmul(
                            ps[:, 0:SQT],
                            lhsT=w_sb[wname][
                                :, j * DQ + g * 128: j * DQ + g * 128 + 128
                            ],
                            rhs=xt[j][:, sq],
                            start=(j == 0),
                            stop=(j == NE - 1),
                        )
                    raw = rtpool.tile([128, SQT], BF16, tag="raw")
                    nc.scalar.copy(raw[:], ps[:, 0:SQT])
                    nc.tensor.matmul(
                        ps[:, SQT:2 * SQT], lhsT=swap_sb[:], rhs=raw[:],
                        start=True, stop=True,
                    )
                    t1 = rtpool.tile([128, SQT], FP32, tag="t1")
                    nc.vector.tensor_mul(t1[:], ps[:, SQT:2 * SQT], sin_sb[:, sq])
                    t2 = rtpool.tile([128, SQT], BF16, tag="t2")
                    nc.vector.tensor_mul(t2[:], raw[:], cos_sb[:, sq])
                    nc.vector.tensor_add(dst[g][:, sq], t1[:], t2[:])

            def attn_chunk(ci):
                sq0 = ci * SQT
                cw = SQT
                nblk = (sq0 + cw) // SKB
                for g in range(2):
                    pso = [
                        psOpool.tile([65, cw], FP32, tag=f"pso{p}",
                                     name=f"pso{p}_{g}_{ci}")
                        for p in range(2)
                    ]
                    pss_tiles = {}

                    def emit_scores(kb):
                        d = max(0, kb * SKB - sq0)
                        pss = psApool.tile([128, 2 * SQT], FP32, tag="psA",
                                           name=f"pss_{g}_{ci}_{kb}")
                        for p in range(2):
                            nc.tensor.matmul(
                                pss[:, p * SQT + d: p * SQT + cw],
                                lhsT=kt_sb[g][
                                    p * 64:(p + 1) * 64,
                                    kb * SKB:(kb + 1) * SKB,
                                ],
                                rhs=qt_sb[g][p * 64:(p + 1) * 64,
                                             sq0 + d: sq0 + cw],
                                start=True,
                                stop=True,
                            )
                        pss_tiles[kb] = (pss, d)

                    emit_scores(0)
                    if nblk > 1:
                        emit_scores(1)
                    for kb in range(nblk):
                        pss, d = pss_tiles.pop(kb)
                        w = cw - d
                        pt = ptpool.tile([128, 2 * SQT], BF16, tag="pt",
                                         name=f"pt_{g}_{ci}_{kb}")
                        # exp over both heads with a (2, w) strided AP
                        pt3 = pt[:].rearrange("p (h c) -> p h c", h=2)[:, :, d:cw]
                        pss3 = pss[:].rearrange("p (h c) -> p h c", h=2)[:, :, d:cw]
                        nc.scalar.activation(
                            pt3, pss3,
                            mybir.ActivationFunctionType.Exp,
                            scale=ATTN_SCALE,
                        )
                        if kb * SKB >= sq0:
                            # partial diagonal 128-col block starts at d
                            for p in range(2):
                                sl = slice(p * SQT + d, p * SQT + d + SKB)
                                nc.vector.tensor_mul(
                                    pt[:, sl], pt[:, sl], mask_sb[:]
                                )
                        for p in range(2):
                            h = 2 * g + p
                            nc.tensor.matmul(
                                pso[p][:, d:cw],
                                lhsT=vaug[kb][:, h * 65:(h + 1) * 65],
                                rhs=pt[:, p * SQT + d: p * SQT + cw],
                                start=(kb == 0),
                                stop=(kb == nblk - 1),
                            )
                        if kb + 2 < nblk:
                            emit_scores(kb + 2)
                    # evacuate pso fast, then the reciprocal chain
                    un = []
                    lrow = []
                    for p in range(2):
                        u = finpool.tile([64, cw], BF16, tag=f"un{p}",
                                         name=f"un{p}_{g}_{ci}")
                        nc.vector.tensor_copy(u[:], pso[p][0:64, :])
                        lr = finpool.tile([1, cw], FP32, tag=f"lrow{p}",
                                          name=f"lrow{p}_{g}_{ci}")
                        nc.vector.tensor_copy(lr[:], pso[p][64:65, :])
                        un.append(u)
                        lrow.append(lr)
                    for p in range(2):
                        h = 2 * g + p
                        linv = finpool.tile([1, cw], FP32, tag=f"linv{p}")
                        nc.vector.reciprocal_approx_fast(linv[:], lrow[p][:])
                        lbc = finpool.tile([64, cw], FP32, tag=f"lbc{p}")
                        nc.gpsimd.partition_broadcast(lbc[:], linv[:])
                        nc.vector.tensor_mul(
                            attnT[:, h * S + sq0: h * S + sq0 + cw],
                            un[p][:], lbc[:],
                        )
                # ---- AllGather this chunk ----
                agin = drampool.tile(
                    [DQ, cw], BF16, tag=f"agin{ci}", name=f"agin{ci}"
                )
                agout = drampool.tile(
                    [E, cw], BF16, tag=f"agout{ci}", name=f"agout{ci}"
                )
                nc.gpsimd.dma_start(
                    out=agin.rearrange("(h p) c -> p h c", h=HPC),
                    in_=attnT[:].rearrange(
                        "p (h s) -> p h s", h=HPC
                    )[:, :, sq0:sq0 + cw],
                )
                nc.gpsimd.collective_compute(
                    "AllGather",
                    mybir.AluOpType.bypass,
                    ins=[agin.opt()],
                    outs=[agout.opt()],
                    replica_groups=REPLICA_GROUPS,
                )
                ag_bufs.append((agout, cw, sq0))

            for st in range(NSQT):
                for g in range(2):
                    qk_block(g, st)
                attn_chunk(st)

            # ---------------- Wo tail ----------------
            for agout, cw, sq0 in ag_bufs:
                gt = gtpool.tile([128, NE * cw], BF16, tag="gtall",
                                 name=f"gt{sq0}")
                nc.sync.dma_start(
                    out=gt[:].rearrange("p (j c) -> p j c", j=NE),
                    in_=agout.rearrange("(j p) c -> p j c", j=NE),
                )
                osb = osbpool.tile([128, (cw // 128) * DQ], FP32, tag="osb",
                                   name=f"osb{sq0}")
                for i4 in range(cw // 128):
                    psw = psWpool.tile([128, DQ], FP32, tag="psw",
                                       name=f"psw{sq0 + i4 * 128}")
                    for j in range(NE):
                        nc.tensor.matmul(
                            psw[:],
                            lhsT=gt[:, j * cw + i4 * 128: j * cw + (i4 + 1) * 128],
                            rhs=w_sb["wo"][:, j * DQ:(j + 1) * DQ],
                            start=(j == 0),
                            stop=(j == NE - 1),
                        )
                    nc.vector.tensor_copy(
                        osb[:, i4 * DQ:(i4 + 1) * DQ], psw[:]
                    )
                nc.scalar.dma_start(
                    out=out_ext[sq0:sq0 + cw, :].rearrange(
                        "(i p) c -> p i c", p=128
                    ),
                    in_=osb[:].rearrange("p (i c) -> p i c", c=DQ),
                )

    nc.finalize()
    return nc


def _host_tables():
    inv = 1.0 / (10000.0 ** (np.arange(0, D, 2, dtype=np.float64) / D))  # (32,)
    ang = np.arange(S, dtype=np.float64)[None, :] * inv[:, None]          # (32,S)
    cos32 = np.cos(ang)
    sin32 = np.sin(ang)
    cos = np.tile(cos32, (4, 1)).astype(np.float32)                       # (128,S)
    sin = np.concatenate([-sin32, sin32, -sin32, sin32], axis=0).astype(np.float32)
    swap = np.zeros((128, 128), np.float32)
    for k in range(128):
        blk = (k // 64) * 64
        swap[k, blk + ((k - blk) + 32) % 64] = 1.0
    # causal triangle for the partial diagonal block: keep col >= row
    tri = (np.arange(128)[None, :] >= np.arange(128)[:, None]).astype(np.float32)
    return cos, sin, swap, tri


def kernel(x, W_q, W_k, W_v, W_o):
    global LAST_RESULT
    if "nc" not in _CACHE:
        _CACHE["nc"] = build_nc()
    nc = _CACHE["nc"]

    bf = ml_dtypes.bfloat16
    perm = np.concatenate([np.arange(0, D, 2), np.arange(1, D, 2)])
    rowperm = (np.arange(H)[:, None] * D + perm[None, :]).reshape(-1)
    Wq_p = W_q[rowperm]
    Wk_p = W_k[rowperm]
    cos, sin, swap, tri = _host_tables()

    in_maps = []
    for c in range(NCORES):
        b, tp = c // TP, c % TP
        sl = slice(tp * DQ, (tp + 1) * DQ)
        in_maps.append({
            "xT": np.ascontiguousarray(x[b].T).astype(bf),
            "wqT": np.ascontiguousarray(Wq_p[sl].T).astype(bf),
            "wkT": np.ascontiguousarray(Wk_p[sl].T).astype(bf),
            "wvT": np.ascontiguousarray(W_v[sl].T).astype(bf),
            "woT": np.ascontiguousarray(W_o[sl].T).astype(bf),
            "cos": cos.astype(bf),
            "sin": sin.astype(bf),
            "swapmat": swap.astype(bf),
            "trimask": tri.astype(bf),
        })

    res = bass_utils.run_bass_kernel_spmd(
        nc, in_maps, core_ids=list(range(NCORES)),
        tmpdir=os.environ.get("BASS_TMPDIR") or None,
    )
    LAST_RESULT = res
    out = np.empty((B, S, E), np.float32)
    for c in range(NCORES):
        b, tp = c // TP, c % TP
        out[b][:, tp * DQ:(tp + 1) * DQ] = np.asarray(
            res.results[c]["out"], dtype=np.float32
        )
    return out


# revision 7
# speedup vs baseline: 1.4695x; 1.1412x over previous
"""Distributed Trainium2 Bass kernel for causal multi-head attention with RoPE.

Reference computation (B=2, S=2048, E=1024, H=16, D=64, fp32):
    q = rope((x @ Wq.T).heads); k = rope((x @ Wk.T).heads); v = (x @ Wv.T).heads
    out = softmax(mask(q k^T / sqrt(E))) v  -> concat heads -> @ Wo.T

Sharding (8 NeuronCores): data parallel over B (2 groups of 4 cores),
tensor parallel over heads within each group (4 heads per core).
Each core computes QKV for its 4 heads, flash-style causal attention,
normalized attention output transposed (d x s), AllGathers per-chunk
across its 4-rank group, then computes a 256-col slice of Wo.

Schedule notes (per-engine queues are in-order; Tile orders by priority):
  - QK projection for seq-block st is issued, then attention chunks in
    that range: exp (ACT) overlaps the PE matmuls of the next block.
  - The kb loop is software-pipelined 3 deep (PSUM pool bufs=3) so the
    PE computes scores kb+1..kb+3 while ACT runs exp(kb).
  - Diagonal score blocks are narrowed to the causal region; the
    remaining partial 128-col block is masked with one triangular
    constant via a DVE multiply.
  - Wo blocks are pushed to the end of the schedule with
    tile_wait_until so the PE never stalls mid-stream on an AllGather.
  - A tiny high-priority warm-up AllGather absorbs the collective
    stream entry barrier + ncfw cold start during phase 1.
  - The last 512 sq block is split into two 256 chunks so the
    tail-exposed final AllGather is half size.

Host-side prep (per-core input shards):
  - x fed transposed (E,S) in bf16.
  - Wq/Wk rows permuted per head to de-interleave RoPE pairs (even dims
    first, odd dims second) so RoPE becomes the rotate-half form.
  - cos/sin tables (bf16), the 32-row swap matrix and the [128,128]
    causal triangle mask are precomputed constants.
"""

import os
import sys

sys.path.insert(0, "/opt/trn_rl_repo")

import numpy as np
import ml_dtypes

import concourse.bass as bass
import concourse.bacc as bacc
import concourse.mybir as mybir
import concourse.tile as tile
from concourse import bass_utils

B, S, E, H, D = 2, 2048, 1024, 16, 64
NCORES = 8
TP = 4                 # tensor-parallel group size
HPC = H // TP          # heads per core = 4
DQ = HPC * D           # per-core projection width = 256
ATTN_SCALE = 1.0 / float(np.sqrt(E))

FP32 = mybir.dt.float32
BF16 = mybir.dt.bfloat16

SQT = 512              # sq block width (qk projection granularity)
SKB = 128              # sk block (partition dim of score tiles)
NSQT = S // SQT        # 4
NST16 = S // 128       # 16
NE = E // 128          # 8 contraction steps

# attention/AllGather chunks: (sq0, cw); last block split to shrink the tail
CHUNKS = [(0, 512), (512, 512), (1024, 512), (1536, 256), (1792, 256)]

REPLICA_GROUPS = [[0, 1, 2, 3], [4, 5, 6, 7]]

_CACHE = {}
LAST_RESULT = None


def build_nc():
    nc = bacc.Bacc(None, target_bir_lowering=False)

    xT = nc.declare_dram_parameter("xT", [E, S], BF16, isOutput=False)
    wqT = nc.declare_dram_parameter("wqT", [E, DQ], BF16, isOutput=False)
    wkT = nc.declare_dram_parameter("wkT", [E, DQ], BF16, isOutput=False)
    wvT = nc.declare_dram_parameter("wvT", [E, DQ], BF16, isOutput=False)
    woT = nc.declare_dram_parameter("woT", [E, DQ], BF16, isOutput=False)
    cosd = nc.declare_dram_parameter("cos", [128, S], BF16, isOutput=False)
    sind = nc.declare_dram_parameter("sin", [128, S], BF16, isOutput=False)
    swapd = nc.declare_dram_parameter("swapmat", [128, 128], BF16, isOutput=False)
    maskd = nc.declare_dram_parameter("trimask", [128, 128], BF16, isOutput=False)
    out_ext = nc.declare_dram_parameter("out", [S, DQ], FP32, isOutput=True)

    with tile.TileContext(nc) as tc:
        with (
            tc.tile_pool(name="dram", bufs=1, space="DRAM") as drampool,
            tc.tile_pool(name="const", bufs=1) as constpool,
            tc.tile_pool(name="psA", bufs=3, space="PSUM") as psApool,
            tc.tile_pool(name="psO", bufs=1, space="PSUM") as psOpool,
            tc.tile_pool(name="ptp", bufs=3) as ptpool,
            tc.tile_pool(name="rope", bufs=3) as rtpool,
            tc.tile_pool(name="fin", bufs=2) as finpool,
            tc.tile_pool(name="gtp", bufs=2) as gtpool,
            tc.tile_pool(name="osb", bufs=2) as osbpool,
        ):
            # ---- persistent SBUF tensors ----
            w_sb = {}
            for name in ("wq", "wk", "wv", "wo"):
                w_sb[name] = constpool.tile(
                    [128, NE * DQ], BF16, tag=f"w_{name}", name=f"w_{name}"
                )
            cos_sb = constpool.tile([128, S], BF16, tag="cos")
            sin_sb = constpool.tile([128, S], BF16, tag="sin")
            swap_sb = constpool.tile([128, 128], BF16, tag="swap")
            mask_sb = constpool.tile([128, 128], BF16, tag="mask")
            xt = [
                constpool.tile([128, S], BF16, tag=f"xT{j}", name=f"xT{j}")
                for j in range(NE)
            ]
            qt_sb = [
                constpool.tile([128, S], BF16, tag=f"qt{g}", name=f"qt{g}")
                for g in range(2)
            ]
            kt_sb = [
                constpool.tile([128, S], BF16, tag=f"kt{g}", name=f"kt{g}")
                for g in range(2)
            ]
            vaug = [
                constpool.tile([128, HPC * 65], BF16, tag=f"vaug{i}", name=f"vaug{i}")
                for i in range(NST16)
            ]
            # all 4 heads' attn^T concatenated along free dim: [:, h*S + s]
            attnT = constpool.tile([64, HPC * S], BF16, tag="attnT", name="attnT")

            def load_w(name, dram):
                # one DMA: w_sb[p, j*DQ + c] = dram[j*128 + p, c]
                nc.sync.dma_start(
                    out=w_sb[name][:].rearrange("p (j c) -> p j c", j=NE),
                    in_=dram.rearrange("(j p) c -> p j c", j=NE),
                )

            # ------- warm-up AllGather: first on the gpsimd queue -------
            warm_in = drampool.tile([64, 8], BF16, tag="warm_in", name="warm_in")
            warm_out = drampool.tile(
                [256, 8], BF16, tag="warm_out", name="warm_out",
            )
            with tc.high_priority():
                nc.gpsimd.collective_compute(
                    "AllGather",
                    mybir.AluOpType.bypass,
                    ins=[warm_in.opt()],
                    outs=[warm_out.opt()],
                    replica_groups=REPLICA_GROUPS,
                )

            # ---------------- input loads (SP queue) ----------------
            load_w("wv", wvT)
            for j in range(NE):
                nc.sync.dma_start(out=xt[j][:], in_=xT[j * 128:(j + 1) * 128, :])
            load_w("wq", wqT)
            load_w("wk", wkT)
            nc.sync.dma_start(out=swap_sb[:], in_=swapd[:])
            nc.sync.dma_start(out=cos_sb[:], in_=cosd[:])
            nc.sync.dma_start(out=sin_sb[:], in_=sind[:])
            nc.sync.dma_start(out=mask_sb[:], in_=maskd[:])
            load_w("wo", woT)

            for i in range(NST16):
                nc.gpsimd.memset(vaug[i][:], 1.0)

            # ---------------- V projection ----------------
            for i in range(NST16):
                psv = psApool.tile([128, 2 * SQT], FP32, tag="psA", name=f"psv{i}")
                for j in range(NE):
                    nc.tensor.matmul(
                        psv[:, 0:DQ],
                        lhsT=xt[j][:, i * 128:(i + 1) * 128],
                        rhs=w_sb["wv"][:, j * DQ:(j + 1) * DQ],
                        start=(j == 0),
                        stop=(j == NE - 1),
                    )
                # one strided copy drops V into the 4 per-head 65-wide
                # slots, leaving column 64 of each slot at 1.0
                nc.vector.tensor_copy(
                    vaug[i][:, 0:HPC * 65].rearrange(
                        "p (h w) -> p h w", h=HPC
                    )[:, :, 0:64],
                    psv[:, 0:DQ].rearrange("p (h w) -> p h w", h=HPC),
                )

            # ---------- interleaved QK projection+RoPE / attention ----------
            ag_bufs = []  # (agout, cw, sq0) in chunk order

            def qk_block(g, st):
                sq = slice(st * SQT, (st + 1) * SQT)
                for wname, dst in (("wq", qt_sb), ("wk", kt_sb)):
                    ps = psApool.tile(
                        [128, 2 * SQT], FP32, tag="psA", name=f"ps_{wname}{g}_{st}"
                    )
                    for j in range(NE):
                        nc.tensor.matmul(
                            ps[:, 0:SQT],
                            lhsT=w_sb[wname][
                                :, j * DQ + g * 128: j * DQ + g * 128 + 128
                            ],
                            rhs=xt[j][:, sq],
                            start=(j == 0),
                            stop=(j == NE - 1),
                        )
                    raw = rtpool.tile([128, SQT], BF16, tag="raw")
                    nc.vector.tensor_copy(raw[:], ps[:, 0:SQT])
                    nc.tensor.matmul(
                        ps[:, SQT:2 * SQT], lhsT=swap_sb[:], rhs=raw[:],
                        start=True, stop=True,
                    )
                    t1 = rtpool.tile([128, SQT], FP32, tag="t1")
                    nc.vector.tensor_mul(t1[:], ps[:, SQT:2 * SQT], sin_sb[:, sq])
                    t2 = rtpool.tile([128, SQT], BF16, tag="t2")
                    nc.vector.tensor_mul(t2[:], raw[:], cos_sb[:, sq])
                    nc.vector.tensor_add(dst[g][:, sq], t1[:], t2[:])

            def attn_chunk(ci):
                sq0, cw = CHUNKS[ci]
                nblk = (sq0 + cw) // SKB
                for g in range(2):
                    pso = [
                        psOpool.tile([65, cw], FP32, tag=f"pso{p}",
                                     name=f"pso{p}_{g}_{ci}")
                        for p in range(2)
                    ]
                    pss_tiles = {}

                    def emit_scores(kb):
                        d = max(0, kb * SKB - sq0)
                        pss = psApool.tile([128, 2 * SQT], FP32, tag="psA",
                                           name=f"pss_{g}_{ci}_{kb}")
                        for p in range(2):
                            nc.tensor.matmul(
                                pss[:, p * SQT + d: p * SQT + cw],
                                lhsT=kt_sb[g][
                                    p * 64:(p + 1) * 64,
                                    kb * SKB:(kb + 1) * SKB,
                                ],
                                rhs=qt_sb[g][p * 64:(p + 1) * 64,
                                             sq0 + d: sq0 + cw],
                                start=True,
                                stop=True,
                            )
                        pss_tiles[kb] = (pss, d)

                    for kb in range(min(3, nblk)):
                        emit_scores(kb)
                    for kb in range(nblk):
                        pss, d = pss_tiles.pop(kb)
                        pt = ptpool.tile([128, 2 * SQT], BF16, tag="pt",
                                         name=f"pt_{g}_{ci}_{kb}")
                        # exp over both heads with a (2, cw-d) strided AP
                        pt3 = pt[:].rearrange("p (h c) -> p h c", h=2)[:, :, d:cw]
                        pss3 = pss[:].rearrange("p (h c) -> p h c", h=2)[:, :, d:cw]
                        nc.scalar.activation(
                            pt3, pss3,
                            mybir.ActivationFunctionType.Exp,
                            scale=ATTN_SCALE,
                        )
                        if kb * SKB >= sq0:
                            # partial diagonal 128-col block starts at d
                            for p in range(2):
                                sl = slice(p * SQT + d, p * SQT + d + SKB)
                                nc.vector.tensor_mul(
                                    pt[:, sl], pt[:, sl], mask_sb[:]
                                )
                        for p in range(2):
                            h = 2 * g + p
                            nc.tensor.matmul(
                                pso[p][:, d:cw],
                                lhsT=vaug[kb][:, h * 65:(h + 1) * 65],
                                rhs=pt[:, p * SQT + d: p * SQT + cw],
                                start=(kb == 0),
                                stop=(kb == nblk - 1),
                            )
                        if kb + 3 < nblk:
                            emit_scores(kb + 3)
                    # evacuate pso fast, then the reciprocal chain
                    un = []
                    lrow = []
                    for p in range(2):
                        lr = finpool.tile([1, cw], FP32, tag=f"lrow{p}",
                                          name=f"lrow{p}_{g}_{ci}")
                        nc.vector.tensor_copy(lr[:], pso[p][64:65, :])
                        u = finpool.tile([64, cw], BF16, tag=f"un{p}",
                                         name=f"un{p}_{g}_{ci}")
                        nc.vector.tensor_copy(u[:], pso[p][0:64, :])
                        un.append(u)
                        lrow.append(lr)
                    for p in range(2):
                        h = 2 * g + p
                        linv = finpool.tile([1, cw], FP32, tag=f"linv{p}")
                        nc.vector.reciprocal_approx_fast(linv[:], lrow[p][:])
                        lbc = finpool.tile([64, cw], FP32, tag=f"lbc{p}")
                        nc.gpsimd.partition_broadcast(lbc[:], linv[:])
                        nc.vector.tensor_mul(
                            attnT[:, h * S + sq0: h * S + sq0 + cw],
                            un[p][:], lbc[:],
                        )
                # ---- AllGather this chunk ----
                agin = drampool.tile(
                    [DQ, cw], BF16, tag=f"agin{ci}", name=f"agin{ci}"
                )
                agout = drampool.tile(
                    [E, cw], BF16, tag=f"agout{ci}", name=f"agout{ci}"
                )
                nc.gpsimd.dma_start(
                    out=agin.rearrange("(h p) c -> p h c", h=HPC),
                    in_=attnT[:].rearrange(
                        "p (h s) -> p h s", h=HPC
                    )[:, :, sq0:sq0 + cw],
                )
                nc.gpsimd.collective_compute(
                    "AllGather",
                    mybir.AluOpType.bypass,
                    ins=[agin.opt()],
                    outs=[agout.opt()],
                    replica_groups=REPLICA_GROUPS,
                )
                ag_bufs.append((agout, cw, sq0))

            ci = 0
            for st in range(NSQT):
                for g in range(2):
                    qk_block(g, st)
                while ci < len(CHUNKS) and (
                    CHUNKS[ci][0] + CHUNKS[ci][1] <= (st + 1) * SQT
                ):
                    attn_chunk(ci)
                    ci += 1

            # ---------------- Wo tail ----------------
            # tile_wait_until pushes these past all attention work in the
            # scheduler's model so the PE stream never blocks on an AG.
            for wi, (agout, cw, sq0) in enumerate(ag_bufs):
                with tc.tile_wait_until(0.5 + 0.02 * wi):
                    gt = gtpool.tile([128, NE * cw], BF16, tag="gtall",
                                     name=f"gt{sq0}")
                    nc.sync.dma_start(
                        out=gt[:].rearrange("p (j c) -> p j c", j=NE),
                        in_=agout.rearrange("(j p) c -> p j c", j=NE),
                    )
                    osb = osbpool.tile([128, (cw // 128) * DQ], FP32, tag="osb",
                                       name=f"osb{sq0}")
                    for i4 in range(cw // 128):
                        psw = psApool.tile([128, 2 * SQT], FP32, tag="psA",
                                           name=f"psw{sq0 + i4 * 128}")
                        for j in range(NE):
                            nc.tensor.matmul(
                                psw[:, 0:DQ],
                                lhsT=gt[:, j * cw + i4 * 128:
                                        j * cw + (i4 + 1) * 128],
                                rhs=w_sb["wo"][:, j * DQ:(j + 1) * DQ],
                                start=(j == 0),
                                stop=(j == NE - 1),
                            )
                        nc.vector.tensor_copy(
                            osb[:, i4 * DQ:(i4 + 1) * DQ], psw[:, 0:DQ]
                        )
                    nc.scalar.dma_start(
                        out=out_ext[sq0:sq0 + cw, :].rearrange(
                            "(i p) c -> p i c", p=128
                        ),
                        in_=osb[:].rearrange("p (i c) -> p i c", c=DQ),
                    )

    nc.finalize()
    return nc


def _host_tables():
    inv = 1.0 / (10000.0 ** (np.arange(0, D, 2, dtype=np.float64) / D))  # (32,)
    ang = np.arange(S, dtype=np.float64)[None, :] * inv[:, None]          # (32,S)
    cos32 = np.cos(ang)
    sin32 = np.sin(ang)
    cos = np.tile(cos32, (4, 1)).astype(np.float32)                       # (128,S)
    sin = np.concatenate([-sin32, sin32, -sin32, sin32], axis=0).astype(np.float32)
    swap = np.zeros((128, 128), np.float32)
    for k in range(128):
        blk = (k // 64) * 64
        swap[k, blk + ((k - blk) + 32) % 64] = 1.0
    # causal triangle for the partial diagonal block: keep col >= row
    tri = (np.arange(128)[None, :] >= np.arange(128)[:, None]).astype(np.float32)
    return cos, sin, swap, tri


def kernel(x, W_q, W_k, W_v, W_o):
    global LAST_RESULT
    if "nc" not in _CACHE:
        _CACHE["nc"] = build_nc()
    nc = _CACHE["nc"]

    bf = ml_dtypes.bfloat16
    perm = np.concatenate([np.arange(0, D, 2), np.arange(1, D, 2)])
    rowperm = (np.arange(H)[:, None] * D + perm[None, :]).reshape(-1)
    Wq_p = W_q[rowperm]
    Wk_p = W_k[rowperm]
    cos, sin, swap, tri = _host_tables()

    in_maps = []
    for c in range(NCORES):
        b, tp = c // TP, c % TP
        sl = slice(tp * DQ, (tp + 1) * DQ)
        in_maps.append({
            "xT": np.ascontiguousarray(x[b].T).astype(bf),
            "wqT": np.ascontiguousarray(Wq_p[sl].T).astype(bf),
            "wkT": np.ascontiguousarray(Wk_p[sl].T).astype(bf),
            "wvT": np.ascontiguousarray(W_v[sl].T).astype(bf),
            "woT": np.ascontiguousarray(W_o[sl].T).astype(bf),
            "cos": cos.astype(bf),
            "sin": sin.astype(bf),
            "swapmat": swap.astype(bf),
            "trimask": tri.astype(bf),
        })

    res = bass_utils.run_bass_kernel_spmd(
        nc, in_maps, core_ids=list(range(NCORES)),
        tmpdir=os.environ.get("BASS_TMPDIR") or None,
    )
    LAST_RESULT = res
    out = np.empty((B, S, E), np.float32)
    for c in range(NCORES):
        b, tp = c // TP, c % TP
        out[b][:, tp * DQ:(tp + 1) * DQ] = np.asarray(
            res.results[c]["out"], dtype=np.float32
        )
    return out
